# revision 14
# baseline (speedup 1.0000x reference)
"""Cross-attention (RMSNorm + QKV proj + 2D RoPE + SDPA + out-proj) on 8
Trainium2 NeuronCores.

Sharding: 8 cores = 4 batches x 2 query-halves. Each core computes the full
KV projection for its batch (duplicated across the 2 cores sharing a batch)
and attention + output projection for its 512 query rows. No collectives.

Layout is feature-major ([feature, seq], features on SBUF partitions);
weights are host-transposed so every linear is one lhsT.T @ rhs PE matmul.
All heavy compute runs in bf16. Inputs arrive as six column-concatenated
DRAM blocks ordered by first use on a single in-order DMA queue (sync) so
the earliest-needed bytes land first; small bias tensors ride the gpsimd
queue so their tiny descriptors never interleave with the wide-row input
stream. A stream of tiny warm-up matmuls at t=0 trips the PE HAM clock
gate before real data lands.

PSUM is hand-placed as one [128, 8, 512] f32 region with phase reuse:
banks 0-5 hold the c-outer K-projection accumulators (6 half-key
accumulators, so the projection rides the kv/wk input stream chunk by
chunk), then become the double-buffered scores tiles (banks 0-3), the
V/Q-chain + denominator-broadcast scratch (banks 4-5), and finally the six
out-projection accumulators. Banks 6-7 alternate as per-head AV
accumulators [65, 512] whose 65th row is the softmax denominator (ones
column appended to V).

The denominator never leaves the chip: the raw den row is copied to SBUF
(bf16), broadcast across 64 partitions with a K=1 matmul per head, and
inverted with the DVE's fast reciprocal approximation; ACT does only exp
(+ RMSNorm ln/exp and drains outside the attention window). V-projection
chains and the remaining Q/K rotation chains are interleaved into early
attention pair slots (deadline-scheduled against a 7-deep AV trail that
drains back to 2 once the chains are done) so they consume PE slack under
the ACT-paced exp pipeline.
"""

import numpy as np

B, SQ, SK, D = 4, 1024, 1024, 768
H, HD = 12, 64
DC = D // 128          # 6 feature chunks
SQL = SQ // 2          # 512 query rows per core
SKC = SK // 128        # 8 key chunks
EPS = 1e-5
NCORES = 8

WE1A = SQL + D          # qT | wq
WE1B = 2 * SQL          # sinQ | cosQ
WF1 = 2 * SK            # sinK | cosK

_cache = {}


# ---------------------------------------------------------------------------
# compiler workarounds
# ---------------------------------------------------------------------------

def _apply_patches():
    """This walrus build allows only ONE sync-wait command per instruction.
    (a) split the Tile kernel-tail drain into one drain per waited proc;
    (b) post-process the BIR JSON, moving excess waits onto same-engine NoOps
    inserted immediately before the over-subscribed instruction."""
    import json
    import concourse.tile as tile
    import concourse.bass as cbass
    from concourse.vector_clock import ScopedClock, VectorClock

    if getattr(cbass.Bass, "_wait_split_patched", False):
        return

    def _drain_and_barrier(self, tick_clock, wait_clock):
        gc = tick_clock.global_clock
        try:
            vec = gc[None]
        except Exception:
            vec = gc
        n = len(vec)
        for p in [i for i in range(n) if vec[i] > 0]:
            sub = [0] * n
            sub[p] = vec[p]
            inst = self.nc.sync.drain()
            wait_clock.add_sem_waits(inst.ins, ScopedClock({None: VectorClock(sub)}))
        self.nc.all_engine_barrier()
        assert self.sems is not None
        popped = self.nc._tile_sem_poison_stack.pop()
        assert popped is self._sem_poison
        self.nc.clear_and_free_semaphores(list(self.sems.allocated().values()))
        self.nc.all_engine_barrier()

    tile.TileContext._drain_and_barrier = _drain_and_barrier

    def _split_waits(bir):
        for f in bir.get("functions", []):
            for blk in f.get("blocks", []):
                insts = blk.get("instructions")
                if not insts:
                    continue
                out = []
                ctr = 0
                for inst in insts:
                    si = inst.get("sync_info")
                    ow = (si or {}).get("on_wait") or []
                    if len(ow) > 1:
                        for w in ow[:-1]:
                            nop = {
                                "name": f"{inst['name']}-ws{ctr}",
                                "opcode": "NoOp",
                                "engine": inst.get("engine"),
                                "ins": [],
                                "outs": [],
                                "sync_info": {"on_wait": [w], "on_update": []},
                            }
                            if "debug" in inst:
                                nop["debug"] = inst["debug"]
                            ctr += 1
                            out.append(nop)
                        si["on_wait"] = [ow[-1]]
                    out.append(inst)
                blk["instructions"] = out
        return bir

    orig = cbass.Bass.to_json_bytes

    def to_json_bytes(self, *a, **kw):
        return json.dumps(_split_waits(json.loads(orig(self, *a, **kw)))).encode()

    cbass.Bass.to_json_bytes = to_json_bytes
    cbass.Bass._wait_split_patched = True


# ---------------------------------------------------------------------------
# device program
# ---------------------------------------------------------------------------

def _build_nc():
    import concourse.bass as bass
    import concourse.tile as tile
    import concourse.mybir as mybir

    F32 = mybir.dt.float32
    BF16 = mybir.dt.bfloat16
    AF = mybir.ActivationFunctionType

    nc = bass.Bass()

    bigA_d = nc.dram_tensor("bigA", [D, SK + D], BF16,
                            kind="ExternalInput")          # kv|wk
    bigE1a_d = nc.dram_tensor("bigE1a", [D, WE1A], BF16,
                              kind="ExternalInput")        # qT|wq
    bigE1b_d = nc.dram_tensor("bigE1b", [D, WE1B], BF16,
                              kind="ExternalInput")        # sinQ|cosQ
    bigF1_d = nc.dram_tensor("bigF1", [D, WF1], BF16,
                             kind="ExternalInput")         # sinK|cosK
    bigE2_d = nc.dram_tensor("bigE2", [D, D], BF16,
                             kind="ExternalInput")         # wv
    bigF2_d = nc.dram_tensor("bigF2", [D, D], BF16,
                             kind="ExternalInput")         # wo
    smalls_d = nc.dram_tensor("smalls", [128, 3 * DC], F32,
                              kind="ExternalInput")        # bq|bk|bo
    out_d = nc.dram_tensor("outT", [D, SQL], BF16, kind="ExternalOutput")

    # all 8 PSUM banks, hand-placed with phase reuse
    ps = nc.alloc_psum_tensor("ps", [128, 8, 512], F32)

    with tile.TileContext(nc) as tc:
        import contextlib
        ctx = contextlib.ExitStack()
        with ctx:
            persist = ctx.enter_context(tc.tile_pool(name="persist", bufs=1))
            tmp = ctx.enter_context(tc.tile_pool(name="tmp", bufs=2))

            # ---- persistent small tensors -------------------------------
            smalls_sb = persist.tile([128, 3 * DC], F32, name="smalls")
            ones_sb = persist.tile([128, 128], BF16, name="ones")
            eps_t = persist.tile([128, 1], F32, name="eps")
            nc.vector.memset(eps_t, EPS)
            nc.vector.memset(ones_sb, 1.0)
            bq_sb = smalls_sb[:, 0:DC]
            bk_sb = smalls_sb[:, DC:2 * DC]
            bo_sb = smalls_sb[:, 2 * DC:3 * DC]

            # ---- persistent activations ---------------------------------
            qn = [persist.tile([128, SQL], BF16, name=f"qn{c}")
                  for c in range(DC)]
            rstd = persist.tile([128, SQL], F32, name="rstd")
            qrot = [persist.tile([128, SQL], BF16, name=f"qrot{c}")
                    for c in range(DC)]
            krot = [persist.tile([128, SK], BF16, name=f"krot{c}")
                    for c in range(DC)]
            vp = [persist.tile([128, H, HD + 1], BF16, name=f"vp{c}")
                  for c in range(SKC)]
            oT = [persist.tile([128, SQL], BF16, name=f"oT{c}")
                  for c in range(DC)]
            kps = [persist.tile([128, SK], BF16, name=f"kps{i}")
                   for i in range(DC)]

            for kc in range(SKC):
                nc.vector.memset(vp[kc][:, :, HD], 1.0)

            nc.gpsimd.dma_start(out=smalls_sb, in_=smalls_d[:, :])

            # ---- PE warm-up: trip the HAM clock gate early --------------
            for i in range(50):
                nc.tensor.matmul(ps[:, 0, 0:128], ones_sb, ones_sb,
                                 start=True, stop=True,
                                 skip_group_check=True)

            # ---- input streams (single in-order queue, first-use order) -
            ab = [persist.tile([128, SK + D], BF16, name=f"ab{c}")
                  for c in range(DC)]
            e1a = [persist.tile([128, WE1A], BF16, name=f"e1a{c}")
                   for c in range(DC)]
            e1b = [persist.tile([128, WE1B], BF16, name=f"e1b{c}")
                   for c in range(DC)]
            fb1 = [persist.tile([128, WF1], BF16, name=f"fb1{c}")
                   for c in range(DC)]
            eb2 = [persist.tile([128, D], BF16, name=f"eb2{c}")
                   for c in range(DC)]
            fb2 = [persist.tile([128, D], BF16, name=f"fb2{c}")
                   for c in range(DC)]
            # chunk-interleaved post order: the critical stream (kv/wk, q/wq,
            # head-0 trig, wv) posts on sync; the late trig chunks (consumed
            # at slots 8-40) and wo post on the gpsimd queue so the sync
            # engine frees up early for the Q-rotation block swaps
            posts = [(ab, bigA_d, c) for c in range(DC)]
            posts += [(e1a, bigE1a_d, c) for c in range(DC)]
            posts += [(fb1, bigF1_d, 0), (e1b, bigE1b_d, 0)]
            posts += [(eb2, bigE2_d, c) for c in range(DC)]
            for tiles, dram, c in posts:
                nc.sync.dma_start(out=tiles[c],
                                  in_=dram[c * 128:(c + 1) * 128, :])
            late = []
            for c in range(1, DC):
                late += [(fb1, bigF1_d, c), (e1b, bigE1b_d, c)]
            late += [(fb2, bigF2_d, c) for c in range(DC)]
            for tiles, dram, c in late:
                nc.gpsimd.dma_start(out=tiles[c],
                                    in_=dram[c * 128:(c + 1) * 128, :])
            kvT = [ab[c][:, 0:SK] for c in range(DC)]
            wk = [ab[c][:, SK:SK + D] for c in range(DC)]
            qT = [e1a[c][:, 0:SQL] for c in range(DC)]
            wq = [e1a[c][:, SQL:SQL + D] for c in range(DC)]
            sinQ = [e1b[c][:, 0:SQL] for c in range(DC)]
            cosQ = [e1b[c][:, SQL:2 * SQL] for c in range(DC)]
            sinK = [fb1[c][:, 0:SK] for c in range(DC)]
            cosK = [fb1[c][:, SK:2 * SK] for c in range(DC)]
            wv = [eb2[c][:, 0:D] for c in range(DC)]
            wo = [fb2[c][:, 0:D] for c in range(DC)]

            # ---- helpers ------------------------------------------------
            def block_swap(dst, src, eng):
                for base in (0, 64):
                    eng.dma_start(out=dst[base:base + 32, :],
                                  in_=src[base + 32:base + 64, :])
                    eng.dma_start(out=dst[base + 32:base + 64, :],
                                  in_=src[base:base + 32, :])

            def combine(p, sin_t, cos_t, dst, eng):
                """rotate drained bf16 projection p [128, 512] into dst."""
                sw = tmp.tile([128, 512], BF16, tag="sw", bufs=3, name="sw")
                block_swap(sw, p, eng)
                t1 = tmp.tile([128, 512], BF16, tag="t1", bufs=3, name="t1")
                nc.vector.tensor_mul(out=t1, in0=sw, in1=sin_t)
                nc.vector.tensor_mul(out=dst, in0=p, in1=cos_t)
                nc.vector.tensor_add(out=dst, in0=dst, in1=t1)

            # ---- K projection, c-outer so it rides the input stream -----
            # banks 0-5 = six m-accumulators over a half of the key range;
            # two passes; ACT drains (idle pre-attention) into bf16 kps
            for half in range(2):
                hs = slice(half * 512, half * 512 + 512)
                for c in range(DC):
                    for m in range(DC):
                        nc.tensor.matmul(
                            ps[:, m, :], wk[c][:, m * 128:(m + 1) * 128],
                            kvT[c][:, hs], start=(c == 0), stop=(c == DC - 1),
                            skip_group_check=True)
                for m in range(DC):
                    nc.scalar.activation(out=kps[m][:, hs], in_=ps[:, m, :],
                                         func=AF.Identity,
                                         bias=bk_sb[:, m:m + 1])

            def kcombine(m):
                for half in range(2):
                    hs = slice(half * 512, half * 512 + 512)
                    combine(kps[m][:, hs], sinK[m][:, hs], cosK[m][:, hs],
                            krot[m][:, hs], nc.gpsimd)

            # ---- V projection (bank-4/5 scratch, drained per half) ------
            def vchain(kc):
                ksl = slice(kc * 128, (kc + 1) * 128)
                for c in range(DC):
                    nc.tensor.matmul(ps[:, 4, :], kvT[c][:, ksl],
                                     wv[c][:, 0:512],
                                     start=(c == 0), stop=(c == DC - 1),
                                     skip_group_check=True)
                    nc.tensor.matmul(ps[:, 5, 0:256], kvT[c][:, ksl],
                                     wv[c][:, 512:768],
                                     start=(c == 0), stop=(c == DC - 1),
                                     skip_group_check=True)
                nc.vector.tensor_copy(
                    out=vp[kc][:, :, 0:HD],
                    in_=ps[:, 4:6, :].rearrange(
                        "p b f -> p (b f)")[:, 0:768].rearrange(
                        "p (h d) -> p h d", h=12))

            # ---- RMSNorm (ss in bank 6, before it becomes AV psum) ------
            for c in range(DC):
                sq = tmp.tile([128, SQL], BF16, tag="sq", bufs=3, name="sq")
                nc.vector.tensor_mul(out=sq, in0=qT[c], in1=qT[c])
                nc.tensor.matmul(ps[:, 6, :], ones_sb, sq, start=(c == 0),
                                 stop=(c == DC - 1), skip_group_check=True)
            ln_t = tmp.tile([128, SQL], F32, tag="lnt", bufs=2, name="lnt")
            nc.scalar.activation(out=ln_t, in_=ps[:, 6, :], func=AF.Ln,
                                 scale=1.0 / D, bias=eps_t)
            nc.scalar.activation(out=rstd, in_=ln_t, func=AF.Exp,
                                 scale=-0.5)
            rstd_bf = persist.tile([128, SQL], BF16, name="rstd_bf")
            nc.vector.tensor_copy(out=rstd_bf, in_=rstd)
            for c in range(DC):
                nc.vector.tensor_mul(out=qn[c], in0=qT[c], in1=rstd_bf)

            def qproj_chain(m):
                for c in range(DC):
                    nc.tensor.matmul(ps[:, 4, :],
                                     wq[c][:, m * 128:(m + 1) * 128], qn[c],
                                     start=(c == 0), stop=(c == DC - 1),
                                     skip_group_check=True)
                p = tmp.tile([128, 512], BF16, tag="p", bufs=3, name="p")
                nc.vector.tensor_scalar_add(out=p, in0=ps[:, 4, :],
                                            scalar1=bq_sb[:, m:m + 1])
                combine(p, sinQ[m], cosQ[m], qrot[m], nc.sync)

            # ---- attention -----------------------------------------------
            po = [ps[0:65, 6, :], ps[0:65, 7, :]]   # per-head AV, alternating
            db = [None] * H

            def emit_av(h, p, e):
                for j in range(2):
                    kc = 2 * p + j
                    nc.tensor.matmul(po[h % 2], vp[kc][:, h, :],
                                     e[:, j, :],
                                     start=(kc == 0), stop=(kc == SKC - 1),
                                     skip_group_check=True)
                if p == 3:
                    finish_head(h)

            def finish_head(h):
                mh, off = h // 2, 64 * (h % 2)
                pot = po[h % 2]
                nc.vector.tensor_copy(out=oT[mh][off:off + 64, :],
                                      in_=pot[0:64, :])
                dbh = tmp.tile([1, SQL], BF16, tag="db", bufs=3, name="db")
                nc.vector.tensor_copy(out=dbh, in_=pot[64:65, :])
                db[h] = dbh
                if h % 2 == 1:
                    # broadcast raw dens across partitions with K=1 matmuls
                    # into bank 4, invert on DVE, normalize
                    nc.tensor.matmul(ps[0:64, 4, :], ones_sb[0:1, 0:64],
                                     db[h - 1], start=True, stop=True,
                                     skip_group_check=True)
                    nc.tensor.matmul(ps[64:128, 4, :], ones_sb[0:1, 0:64],
                                     db[h], start=True, stop=True,
                                     skip_group_check=True)
                    lnb = tmp.tile([128, SQL], F32, tag="lnb", bufs=2,
                                   name="lnb")
                    nc.scalar.activation(out=lnb, in_=ps[:, 4, :],
                                         func=AF.Ln)
                    rr = tmp.tile([128, SQL], BF16, tag="rr", bufs=2,
                                  name="rr")
                    nc.scalar.activation(out=rr, in_=lnb, func=AF.Exp,
                                         scale=-1.0)
                    nc.vector.tensor_mul(out=oT[mh], in0=oT[mh], in1=rr)

            qproj_chain(0)
            kcombine(0)

            # bridge fillers: keep HAM warm across the small PE idle gap
            # between the projection phase and the first scores
            for i in range(48):
                nc.tensor.matmul(ps[:, 0, 0:128], ones_sb, ones_sb,
                                 start=True, stop=True,
                                 skip_group_check=True)

            # deferred chains, deadline-scheduled into early pair slots
            chains = {1: lambda: vchain(0), 2: lambda: vchain(1),
                      3: lambda: vchain(2), 4: lambda: vchain(3),
                      5: lambda: vchain(4), 6: lambda: vchain(5),
                      7: lambda: qproj_chain(1), 8: lambda: kcombine(1),
                      9: lambda: vchain(6), 10: lambda: vchain(7),
                      11: lambda: qproj_chain(2), 12: lambda: kcombine(2),
                      13: lambda: qproj_chain(3), 14: lambda: kcombine(3),
                      15: lambda: qproj_chain(4), 16: lambda: kcombine(4),
                      17: lambda: qproj_chain(5), 18: lambda: kcombine(5)}

            pend = []
            slot = 0
            for h in range(H):
                mh, off = h // 2, 64 * (h % 2)
                for p in range(4):
                    if slot in chains:
                        chains[slot]()
                    # scores double-buffer: banks 0-1 / 2-3
                    sb = 2 * (slot % 2)
                    for j in range(2):
                        kc = 2 * p + j
                        nc.tensor.matmul(
                            ps[:, sb + j, :],
                            krot[mh][off:off + 64, kc * 128:(kc + 1) * 128],
                            qrot[mh][off:off + 64, :],
                            start=True, stop=True, skip_group_check=True)
                    e = tmp.tile([128, 2, 512], BF16, tag="ex", bufs=9,
                                 name="ex")
                    nc.scalar.activation(out=e, in_=ps[:, sb:sb + 2, :],
                                         func=AF.Exp, scale=0.125)
                    thr = 7 if slot < 19 else max(2, 7 - (slot - 19) // 2)
                    while len(pend) >= thr:
                        emit_av(*pend.pop(0))
                    pend.append((h, p, e))
                    slot += 1
            for u in pend:
                emit_av(*u)

            # ---- output projection (c-outer, 6 psum accumulators) -------
            for c in range(DC - 1):
                for m in range(DC):
                    nc.tensor.matmul(ps[:, m, :],
                                     wo[c][:, m * 128:(m + 1) * 128],
                                     oT[c], start=(c == 0), stop=False,
                                     skip_group_check=True)

            def c5_mm(m):
                nc.tensor.matmul(ps[:, m, :],
                                 wo[DC - 1][:, m * 128:(m + 1) * 128],
                                 oT[DC - 1], start=False, stop=True,
                                 skip_group_check=True)

            def c5_drain(m):
                outc = tmp.tile([128, SQL], BF16, tag="outb", bufs=3,
                                name="outc")
                nc.scalar.activation(out=outc, in_=ps[:, m, :],
                                     func=AF.Identity,
                                     bias=bo_sb[:, m:m + 1])
                nc.sync.dma_start(out=out_d[m * 128:(m + 1) * 128, :],
                                  in_=outc)

            c5_mm(0)
            for m in range(1, DC):
                c5_mm(m)
                c5_drain(m - 1)
            c5_drain(DC - 1)

    return nc


# ---------------------------------------------------------------------------
# host wrapper
# ---------------------------------------------------------------------------

def kernel(q, kv, posq, posk, w_norm, w_q, b_q, w_kv, b_kv, w_out, b_out, freqs):
    _apply_patches()
    import ml_dtypes
    from concourse.bass_utils import run_bass_kernel_spmd

    BF = ml_dtypes.bfloat16

    q = np.asarray(q, np.float32)
    kv = np.asarray(kv, np.float32)
    posq_np = np.asarray(posq)
    posk_np = np.asarray(posk)
    w_norm = np.asarray(w_norm, np.float32)
    w_q = np.asarray(w_q, np.float32)
    b_q = np.asarray(b_q, np.float32)
    w_kv = np.asarray(w_kv, np.float32)
    b_kv = np.asarray(b_kv, np.float32)
    w_out = np.asarray(w_out, np.float32)
    b_out = np.asarray(b_out, np.float32)
    freqs = np.asarray(freqs, np.float32)

    # de-interleave head dims: new j<32 -> old 2j (even), j>=32 -> old 2(j-32)+1
    perm = np.empty(D, np.int64)
    for h in range(H):
        for j in range(HD):
            perm[h * HD + j] = h * HD + (2 * j if j < 32 else 2 * (j - 32) + 1)

    wqT = np.ascontiguousarray((w_q[perm, :] * w_norm[None, :]).T).astype(BF)
    wkT = np.ascontiguousarray(w_kv[:D][perm, :].T).astype(BF)
    wvT = np.ascontiguousarray(w_kv[D:].T).astype(BF)
    woT = np.ascontiguousarray(w_out.T).astype(BF)
    bqR = np.ascontiguousarray(b_q[perm].reshape(DC, 128).T)
    bkR = np.ascontiguousarray(b_kv[:D][perm].reshape(DC, 128).T)
    bo_eff = b_out + w_out @ b_kv[D:]          # fold V bias (softmax sums to 1)
    boR = np.ascontiguousarray(bo_eff.reshape(DC, 128).T)
    smalls = np.ascontiguousarray(
        np.concatenate([bqR, bkR, boR], axis=1).astype(np.float32))

    # RoPE sin/cos tables in de-interleaved feature-major layout, with the
    # rotation sign folded into the sin table (e-rows negated)
    fr = np.empty((2, D), np.float64)
    for h in range(H):
        f = freqs[:, h, :].astype(np.float64)   # [2, 32]
        fr[:, h * HD:h * HD + 32] = -f
        fr[:, h * HD + 32:(h + 1) * HD] = f

    def trig_tables(pos2):  # pos [S, 2] int -> sinT, cosT [D, S] bf16
        ang = fr.T @ pos2.T.astype(np.float64)  # [D, S]
        return np.sin(ang).astype(BF), np.cos(ang).astype(BF)

    if "nc" not in _cache:
        _cache["nc"] = _build_nc()
    nc = _cache["nc"]

    bigA_b = []
    bigF1_b = []
    for b in range(B):
        kvb = kv[b].T.astype(BF)
        sK, cK = trig_tables(posk_np[b])
        bigA_b.append(np.ascontiguousarray(
            np.concatenate([kvb, wkT], axis=1)))
        bigF1_b.append(np.ascontiguousarray(
            np.concatenate([sK, cK], axis=1)))

    in_maps = []
    for core in range(NCORES):
        b, half = core // 2, core % 2
        qs = slice(half * SQL, (half + 1) * SQL)
        sQ, cQ = trig_tables(posq_np[b, qs, :])
        bigE1a = np.ascontiguousarray(
            np.concatenate([q[b, qs, :].T.astype(BF), wqT], axis=1))
        bigE1b = np.ascontiguousarray(np.concatenate([sQ, cQ], axis=1))
        in_maps.append({
            "bigA": bigA_b[b], "bigE1a": bigE1a, "bigE1b": bigE1b,
            "bigF1": bigF1_b[b], "bigE2": wvT, "bigF2": woT,
            "smalls": smalls,
        })

    res = run_bass_kernel_spmd(nc, in_maps, core_ids=list(range(NCORES)))
    kernel._last_result = res

    out = np.empty((B, SQ, D), np.float32)
    for core in range(NCORES):
        b, half = core // 2, core % 2
        out[b, half * SQL:(half + 1) * SQL, :] = \
            res.results[core]["outT"].T.astype(np.float32)
    return out


# revision 15
# speedup vs baseline: 1.1928x; 1.1928x over previous
"""Cross-attention (RMSNorm + QKV proj + 2D RoPE + SDPA + out-proj) on 8
Trainium2 NeuronCores.

Sharding: 8 cores = 4 batches x 2 query-halves. Each core computes the full
KV projection for its batch (duplicated across the 2 cores sharing a batch)
and attention + output projection for its 512 query rows. No collectives.

Layout is feature-major ([feature, seq], features on SBUF partitions);
weights are host-transposed so every linear is one lhsT.T @ rhs PE matmul.
All heavy compute runs in bf16. Inputs arrive as six column-concatenated
DRAM blocks ordered by first use on a single in-order DMA queue (sync) so
the earliest-needed bytes land first; small bias tensors ride the gpsimd
queue so their tiny descriptors never interleave with the wide-row input
stream. A stream of tiny warm-up matmuls at t=0 trips the PE HAM clock
gate before real data lands.

PSUM is hand-placed as one [128, 8, 512] f32 region with phase reuse:
banks 0-5 hold the c-outer K-projection accumulators (6 half-key
accumulators, so the projection rides the kv/wk input stream chunk by
chunk), then become the double-buffered scores tiles (banks 0-3), the
V/Q-chain + denominator-broadcast scratch (banks 4-5), and finally the six
out-projection accumulators. Banks 6-7 alternate as per-head AV
accumulators [65, 512] whose 65th row is the softmax denominator (ones
column appended to V).

The denominator never leaves the chip: the raw den row is copied to SBUF
(bf16), broadcast across 64 partitions with a K=1 matmul per head, and
inverted with the DVE's fast reciprocal approximation; ACT does only exp
(+ RMSNorm ln/exp and drains outside the attention window). V-projection
chains and the remaining Q/K rotation chains are interleaved into early
attention pair slots (deadline-scheduled against a 7-deep AV trail that
drains back to 2 once the chains are done) so they consume PE slack under
the ACT-paced exp pipeline.
"""

import numpy as np

B, SQ, SK, D = 4, 1024, 1024, 768
H, HD = 12, 64
DC = D // 128          # 6 feature chunks
SQL = SQ // 2          # 512 query rows per core
SKC = SK // 128        # 8 key chunks
EPS = 1e-5
NCORES = 8

WE1A = SQL + D          # qT | wq
WE1B = 2 * SQL          # sinQ | cosQ
WF1 = 2 * SK            # sinK | cosK

_cache = {}


# ---------------------------------------------------------------------------
# compiler workarounds
# ---------------------------------------------------------------------------

def _apply_patches():
    """This walrus build allows only ONE sync-wait command per instruction.
    (a) split the Tile kernel-tail drain into one drain per waited proc;
    (b) post-process the BIR JSON, moving excess waits onto same-engine NoOps
    inserted immediately before the over-subscribed instruction."""
    import json
    import concourse.tile as tile
    import concourse.bass as cbass
    from concourse.vector_clock import ScopedClock, VectorClock

    if getattr(cbass.Bass, "_wait_split_patched", False):
        return

    def _drain_and_barrier(self, tick_clock, wait_clock):
        gc = tick_clock.global_clock
        try:
            vec = gc[None]
        except Exception:
            vec = gc
        n = len(vec)
        for p in [i for i in range(n) if vec[i] > 0]:
            sub = [0] * n
            sub[p] = vec[p]
            inst = self.nc.sync.drain()
            wait_clock.add_sem_waits(inst.ins, ScopedClock({None: VectorClock(sub)}))
        self.nc.all_engine_barrier()
        assert self.sems is not None
        popped = self.nc._tile_sem_poison_stack.pop()
        assert popped is self._sem_poison
        self.nc.clear_and_free_semaphores(list(self.sems.allocated().values()))
        self.nc.all_engine_barrier()

    tile.TileContext._drain_and_barrier = _drain_and_barrier

    def _split_waits(bir):
        for f in bir.get("functions", []):
            for blk in f.get("blocks", []):
                insts = blk.get("instructions")
                if not insts:
                    continue
                out = []
                ctr = 0
                for inst in insts:
                    si = inst.get("sync_info")
                    ow = (si or {}).get("on_wait") or []
                    if len(ow) > 1:
                        for w in ow[:-1]:
                            nop = {
                                "name": f"{inst['name']}-ws{ctr}",
                                "opcode": "NoOp",
                                "engine": inst.get("engine"),
                                "ins": [],
                                "outs": [],
                                "sync_info": {"on_wait": [w], "on_update": []},
                            }
                            if "debug" in inst:
                                nop["debug"] = inst["debug"]
                            ctr += 1
                            out.append(nop)
                        si["on_wait"] = [ow[-1]]
                    out.append(inst)
                blk["instructions"] = out
        return bir

    orig = cbass.Bass.to_json_bytes

    def to_json_bytes(self, *a, **kw):
        return json.dumps(_split_waits(json.loads(orig(self, *a, **kw)))).encode()

    cbass.Bass.to_json_bytes = to_json_bytes
    cbass.Bass._wait_split_patched = True


# ---------------------------------------------------------------------------
# device program
# ---------------------------------------------------------------------------

def _build_nc():
    import concourse.bass as bass
    import concourse.tile as tile
    import concourse.mybir as mybir

    F32 = mybir.dt.float32
    BF16 = mybir.dt.bfloat16
    AF = mybir.ActivationFunctionType

    nc = bass.Bass()

    bigA_d = nc.dram_tensor("bigA", [D, SK + D], BF16,
                            kind="ExternalInput")          # kv|wk
    bigE1a_d = nc.dram_tensor("bigE1a", [D, WE1A], BF16,
                              kind="ExternalInput")        # qT|wq
    bigT_d = nc.dram_tensor("bigT", [D, WF1 + WE1B], BF16,
                            kind="ExternalInput")          # sinK|cosK|sinQ|cosQ
    bigE2_d = nc.dram_tensor("bigE2", [D, D], BF16,
                             kind="ExternalInput")         # wv
    bigF2_d = nc.dram_tensor("bigF2", [D, D], BF16,
                             kind="ExternalInput")         # wo
    smalls_d = nc.dram_tensor("smalls", [128, 3 * DC], F32,
                              kind="ExternalInput")        # bq|bk|bo
    out_d = nc.dram_tensor("outT", [D, SQL], BF16, kind="ExternalOutput")

    # all 8 PSUM banks, hand-placed with phase reuse
    ps = nc.alloc_psum_tensor("ps", [128, 8, 512], F32)

    with tile.TileContext(nc) as tc:
        import contextlib
        ctx = contextlib.ExitStack()
        with ctx:
            persist = ctx.enter_context(tc.tile_pool(name="persist", bufs=1))
            tmp = ctx.enter_context(tc.tile_pool(name="tmp", bufs=2))

            # ---- persistent small tensors -------------------------------
            smalls_sb = persist.tile([128, 3 * DC], F32, name="smalls")
            ones_sb = persist.tile([128, 128], BF16, name="ones")
            eps_t = persist.tile([128, 1], F32, name="eps")
            nc.vector.memset(eps_t, EPS)
            nc.vector.memset(ones_sb, 1.0)
            bq_sb = smalls_sb[:, 0:DC]
            bk_sb = smalls_sb[:, DC:2 * DC]
            bo_sb = smalls_sb[:, 2 * DC:3 * DC]

            # ---- persistent activations ---------------------------------
            qn = [persist.tile([128, SQL], BF16, name=f"qn{c}")
                  for c in range(DC)]
            rstd = persist.tile([128, SQL], F32, name="rstd")
            qrot = [persist.tile([128, SQL], BF16, name=f"qrot{c}")
                    for c in range(DC)]
            krot = [persist.tile([128, SK], BF16, name=f"krot{c}")
                    for c in range(DC)]
            vp = [persist.tile([128, H, HD + 1], BF16, name=f"vp{c}")
                  for c in range(SKC)]
            oT = [persist.tile([128, SQL], BF16, name=f"oT{c}")
                  for c in range(DC)]
            kps = [persist.tile([128, SK], BF16, name=f"kps{i}")
                   for i in range(DC)]

            for kc in range(SKC):
                nc.vector.memset(vp[kc][:, :, HD], 1.0)

            nc.gpsimd.dma_start(out=smalls_sb, in_=smalls_d[:, :])

            # ---- PE warm-up: trip the HAM clock gate early --------------
            for i in range(50):
                nc.tensor.matmul(ps[:, 0, 0:128], ones_sb, ones_sb,
                                 start=True, stop=True,
                                 skip_group_check=True)

            # ---- input streams (single in-order queue, first-use order) -
            ab = [persist.tile([128, SK + D], BF16, name=f"ab{c}")
                  for c in range(DC)]
            g1e = persist.tile([128, DC, WE1A], BF16, name="g1e")
            tg = [persist.tile([128, 4 * SK // 2 + 0 + WE1B], BF16,
                               name=f"tg{c}") for c in range(DC)]
            eb2 = [persist.tile([128, D], BF16, name=f"eb2{c}")
                   for c in range(DC)]
            fb2 = [persist.tile([128, D], BF16, name=f"fb2{c}")
                   for c in range(DC)]
            # post order: kv/wk chunks (K-proj rides them), then q/wq as one
            # merged 6-chunk DMA, head-0 trig, wv, the late trig chunks, wo.
            # Merging cuts sync posting serialization (36 -> 15 posts).
            for c in range(DC):
                nc.sync.dma_start(out=ab[c],
                                  in_=bigA_d[c * 128:(c + 1) * 128, :])
            nc.sync.dma_start(
                out=g1e,
                in_=bigE1a_d.rearrange("(c p) w -> p c w", p=128))
            nc.sync.dma_start(out=tg[0], in_=bigT_d[0:128, :])
            for c in range(DC):
                nc.sync.dma_start(out=eb2[c],
                                  in_=bigE2_d[c * 128:(c + 1) * 128, :])
            for c in range(1, DC):
                nc.sync.dma_start(out=tg[c],
                                  in_=bigT_d[c * 128:(c + 1) * 128, :])
            for c in range(DC):
                nc.sync.dma_start(out=fb2[c],
                                  in_=bigF2_d[c * 128:(c + 1) * 128, :])
            kvT = [ab[c][:, 0:SK] for c in range(DC)]
            wk = [ab[c][:, SK:SK + D] for c in range(DC)]
            qT = [g1e[:, c, 0:SQL] for c in range(DC)]
            wq = [g1e[:, c, SQL:SQL + D] for c in range(DC)]
            sinK = [tg[c][:, 0:SK] for c in range(DC)]
            cosK = [tg[c][:, SK:2 * SK] for c in range(DC)]
            sinQ = [tg[c][:, 2 * SK:2 * SK + SQL] for c in range(DC)]
            cosQ = [tg[c][:, 2 * SK + SQL:2 * SK + 2 * SQL]
                    for c in range(DC)]
            wv = [eb2[c][:, 0:D] for c in range(DC)]
            wo = [fb2[c][:, 0:D] for c in range(DC)]

            # ---- helpers ------------------------------------------------
            def block_swap(dst, src, eng):
                for base in (0, 64):
                    eng.dma_start(out=dst[base:base + 32, :],
                                  in_=src[base + 32:base + 64, :])
                    eng.dma_start(out=dst[base + 32:base + 64, :],
                                  in_=src[base:base + 32, :])

            def combine(p, sin_t, cos_t, dst, eng):
                """rotate drained bf16 projection p [128, 512] into dst."""
                sw = tmp.tile([128, 512], BF16, tag="sw", bufs=3, name="sw")
                block_swap(sw, p, eng)
                t1 = tmp.tile([128, 512], BF16, tag="t1", bufs=3, name="t1")
                nc.vector.tensor_mul(out=t1, in0=sw, in1=sin_t)
                nc.vector.tensor_mul(out=dst, in0=p, in1=cos_t)
                nc.vector.tensor_add(out=dst, in0=dst, in1=t1)

            # ---- K projection, c-outer so it rides the input stream -----
            # banks 0-5 = six m-accumulators over a half of the key range;
            # two passes; ACT drains (idle pre-attention) into bf16 kps
            for half in range(2):
                hs = slice(half * 512, half * 512 + 512)
                for c in range(DC):
                    for m in range(DC):
                        nc.tensor.matmul(
                            ps[:, m, :], wk[c][:, m * 128:(m + 1) * 128],
                            kvT[c][:, hs], start=(c == 0), stop=(c == DC - 1),
                            skip_group_check=True)
                for m in range(DC):
                    nc.scalar.activation(out=kps[m][:, hs], in_=ps[:, m, :],
                                         func=AF.Identity,
                                         bias=bk_sb[:, m:m + 1])

            def kcombine(m):
                for half in range(2):
                    hs = slice(half * 512, half * 512 + 512)
                    combine(kps[m][:, hs], sinK[m][:, hs], cosK[m][:, hs],
                            krot[m][:, hs], nc.gpsimd)

            # ---- V projection (bank-4/5 scratch, drained per half) ------
            def vchain(kc):
                ksl = slice(kc * 128, (kc + 1) * 128)
                for c in range(DC):
                    nc.tensor.matmul(ps[:, 4, :], kvT[c][:, ksl],
                                     wv[c][:, 0:512],
                                     start=(c == 0), stop=(c == DC - 1),
                                     skip_group_check=True)
                    nc.tensor.matmul(ps[:, 5, 0:256], kvT[c][:, ksl],
                                     wv[c][:, 512:768],
                                     start=(c == 0), stop=(c == DC - 1),
                                     skip_group_check=True)
                nc.vector.tensor_copy(
                    out=vp[kc][:, :, 0:HD],
                    in_=ps[:, 4:6, :].rearrange(
                        "p b f -> p (b f)")[:, 0:768].rearrange(
                        "p (h d) -> p h d", h=12))

            # ---- RMSNorm (ss in bank 6, before it becomes AV psum) ------
            for c in range(DC):
                sq = tmp.tile([128, SQL], BF16, tag="sq", bufs=3, name="sq")
                nc.vector.tensor_mul(out=sq, in0=qT[c], in1=qT[c])
                nc.tensor.matmul(ps[:, 6, :], ones_sb, sq, start=(c == 0),
                                 stop=(c == DC - 1), skip_group_check=True)
            ln_t = tmp.tile([128, SQL], F32, tag="lnt", bufs=2, name="lnt")
            nc.scalar.activation(out=ln_t, in_=ps[:, 6, :], func=AF.Ln,
                                 scale=1.0 / D, bias=eps_t)
            nc.scalar.activation(out=rstd, in_=ln_t, func=AF.Exp,
                                 scale=-0.5)
            rstd_bf = persist.tile([128, SQL], BF16, name="rstd_bf")
            nc.vector.tensor_copy(out=rstd_bf, in_=rstd)
            for c in range(DC):
                nc.vector.tensor_mul(out=qn[c], in0=qT[c], in1=rstd_bf)

            def qproj_chain(m):
                for c in range(DC):
                    nc.tensor.matmul(ps[:, 4, :],
                                     wq[c][:, m * 128:(m + 1) * 128], qn[c],
                                     start=(c == 0), stop=(c == DC - 1),
                                     skip_group_check=True)
                p = tmp.tile([128, 512], BF16, tag="p", bufs=3, name="p")
                nc.vector.tensor_scalar_add(out=p, in0=ps[:, 4, :],
                                            scalar1=bq_sb[:, m:m + 1])
                combine(p, sinQ[m], cosQ[m], qrot[m], nc.sync)

            # ---- attention -----------------------------------------------
            po = [ps[0:65, 6, :], ps[0:65, 7, :]]   # per-head AV, alternating
            db = [None] * H

            def emit_av(h, p, e):
                for j in range(2):
                    kc = 2 * p + j
                    nc.tensor.matmul(po[h % 2], vp[kc][:, h, :],
                                     e[:, j, :],
                                     start=(kc == 0), stop=(kc == SKC - 1),
                                     skip_group_check=True)
                if p == 3:
                    finish_head(h)

            def finish_head(h):
                mh, off = h // 2, 64 * (h % 2)
                pot = po[h % 2]
                nc.vector.tensor_copy(out=oT[mh][off:off + 64, :],
                                      in_=pot[0:64, :])
                dbh = tmp.tile([1, SQL], BF16, tag="db", bufs=3, name="db")
                nc.vector.tensor_copy(out=dbh, in_=pot[64:65, :])
                db[h] = dbh
                if h % 2 == 1:
                    # broadcast raw dens across partitions with K=1 matmuls
                    # into bank 4, invert on DVE, normalize
                    nc.tensor.matmul(ps[0:64, 4, :], ones_sb[0:1, 0:64],
                                     db[h - 1], start=True, stop=True,
                                     skip_group_check=True)
                    nc.tensor.matmul(ps[64:128, 4, :], ones_sb[0:1, 0:64],
                                     db[h], start=True, stop=True,
                                     skip_group_check=True)
                    lnb = tmp.tile([128, SQL], F32, tag="lnb", bufs=2,
                                   name="lnb")
                    nc.scalar.activation(out=lnb, in_=ps[:, 4, :],
                                         func=AF.Ln)
                    rr = tmp.tile([128, SQL], BF16, tag="rr", bufs=2,
                                  name="rr")
                    nc.scalar.activation(out=rr, in_=lnb, func=AF.Exp,
                                         scale=-1.0)
                    nc.vector.tensor_mul(out=oT[mh], in0=oT[mh], in1=rr)

            qproj_chain(0)
            kcombine(0)

            # bridge fillers: keep HAM warm across the small PE idle gap
            # between the projection phase and the first scores
            for i in range(48):
                nc.tensor.matmul(ps[:, 0, 0:128], ones_sb, ones_sb,
                                 start=True, stop=True,
                                 skip_group_check=True)

            # deferred chains, deadline-scheduled into early pair slots
            chains = {1: lambda: vchain(0), 2: lambda: vchain(1),
                      3: lambda: vchain(2), 4: lambda: vchain(3),
                      5: lambda: vchain(4), 6: lambda: vchain(5),
                      7: lambda: qproj_chain(1), 8: lambda: kcombine(1),
                      9: lambda: vchain(6), 10: lambda: vchain(7),
                      11: lambda: qproj_chain(2), 12: lambda: kcombine(2),
                      13: lambda: qproj_chain(3), 14: lambda: kcombine(3),
                      15: lambda: qproj_chain(4), 16: lambda: kcombine(4),
                      17: lambda: qproj_chain(5), 18: lambda: kcombine(5)}

            pend = []
            slot = 0
            for h in range(H):
                mh, off = h // 2, 64 * (h % 2)
                for p in range(4):
                    if slot in chains:
                        chains[slot]()
                    # scores double-buffer: banks 0-1 / 2-3
                    sb = 2 * (slot % 2)
                    for j in range(2):
                        kc = 2 * p + j
                        nc.tensor.matmul(
                            ps[:, sb + j, :],
                            krot[mh][off:off + 64, kc * 128:(kc + 1) * 128],
                            qrot[mh][off:off + 64, :],
                            start=True, stop=True, skip_group_check=True)
                    e = tmp.tile([128, 2, 512], BF16, tag="ex", bufs=9,
                                 name="ex")
                    nc.scalar.activation(out=e, in_=ps[:, sb:sb + 2, :],
                                         func=AF.Exp, scale=0.125)
                    thr = 7 if slot < 19 else max(2, 7 - (slot - 19) // 2)
                    while len(pend) >= thr:
                        emit_av(*pend.pop(0))
                    pend.append((h, p, e))
                    slot += 1
            for u in pend:
                emit_av(*u)

            # ---- output projection (c-outer, 6 psum accumulators) -------
            for c in range(DC - 1):
                for m in range(DC):
                    nc.tensor.matmul(ps[:, m, :],
                                     wo[c][:, m * 128:(m + 1) * 128],
                                     oT[c], start=(c == 0), stop=False,
                                     skip_group_check=True)

            def c5_mm(m):
                nc.tensor.matmul(ps[:, m, :],
                                 wo[DC - 1][:, m * 128:(m + 1) * 128],
                                 oT[DC - 1], start=False, stop=True,
                                 skip_group_check=True)

            def c5_drain(m):
                outc = tmp.tile([128, SQL], BF16, tag="outb", bufs=3,
                                name="outc")
                nc.scalar.activation(out=outc, in_=ps[:, m, :],
                                     func=AF.Identity,
                                     bias=bo_sb[:, m:m + 1])
                nc.sync.dma_start(out=out_d[m * 128:(m + 1) * 128, :],
                                  in_=outc)

            c5_mm(0)
            for m in range(1, DC):
                c5_mm(m)
                c5_drain(m - 1)
            c5_drain(DC - 1)

    return nc


# ---------------------------------------------------------------------------
# host wrapper
# ---------------------------------------------------------------------------

def kernel(q, kv, posq, posk, w_norm, w_q, b_q, w_kv, b_kv, w_out, b_out, freqs):
    _apply_patches()
    import ml_dtypes
    from concourse.bass_utils import run_bass_kernel_spmd

    BF = ml_dtypes.bfloat16

    q = np.asarray(q, np.float32)
    kv = np.asarray(kv, np.float32)
    posq_np = np.asarray(posq)
    posk_np = np.asarray(posk)
    w_norm = np.asarray(w_norm, np.float32)
    w_q = np.asarray(w_q, np.float32)
    b_q = np.asarray(b_q, np.float32)
    w_kv = np.asarray(w_kv, np.float32)
    b_kv = np.asarray(b_kv, np.float32)
    w_out = np.asarray(w_out, np.float32)
    b_out = np.asarray(b_out, np.float32)
    freqs = np.asarray(freqs, np.float32)

    # de-interleave head dims: new j<32 -> old 2j (even), j>=32 -> old 2(j-32)+1
    perm = np.empty(D, np.int64)
    for h in range(H):
        for j in range(HD):
            perm[h * HD + j] = h * HD + (2 * j if j < 32 else 2 * (j - 32) + 1)

    wqT = np.ascontiguousarray((w_q[perm, :] * w_norm[None, :]).T).astype(BF)
    wkT = np.ascontiguousarray(w_kv[:D][perm, :].T).astype(BF)
    wvT = np.ascontiguousarray(w_kv[D:].T).astype(BF)
    woT = np.ascontiguousarray(w_out.T).astype(BF)
    bqR = np.ascontiguousarray(b_q[perm].reshape(DC, 128).T)
    bkR = np.ascontiguousarray(b_kv[:D][perm].reshape(DC, 128).T)
    bo_eff = b_out + w_out @ b_kv[D:]          # fold V bias (softmax sums to 1)
    boR = np.ascontiguousarray(bo_eff.reshape(DC, 128).T)
    smalls = np.ascontiguousarray(
        np.concatenate([bqR, bkR, boR], axis=1).astype(np.float32))

    # RoPE sin/cos tables in de-interleaved feature-major layout, with the
    # rotation sign folded into the sin table (e-rows negated)
    fr = np.empty((2, D), np.float64)
    for h in range(H):
        f = freqs[:, h, :].astype(np.float64)   # [2, 32]
        fr[:, h * HD:h * HD + 32] = -f
        fr[:, h * HD + 32:(h + 1) * HD] = f

    def trig_tables(pos2):  # pos [S, 2] int -> sinT, cosT [D, S] bf16
        ang = fr.T @ pos2.T.astype(np.float64)  # [D, S]
        return np.sin(ang).astype(BF), np.cos(ang).astype(BF)

    if "nc" not in _cache:
        _cache["nc"] = _build_nc()
    nc = _cache["nc"]

    bigA_b = []
    kf_b = []
    for b in range(B):
        kvb = kv[b].T.astype(BF)
        sK, cK = trig_tables(posk_np[b])
        bigA_b.append(np.ascontiguousarray(
            np.concatenate([kvb, wkT], axis=1)))
        kf_b.append((sK, cK))

    in_maps = []
    for core in range(NCORES):
        b, half = core // 2, core % 2
        qs = slice(half * SQL, (half + 1) * SQL)
        sQ, cQ = trig_tables(posq_np[b, qs, :])
        sK, cK = kf_b[b]
        bigE1a = np.ascontiguousarray(
            np.concatenate([q[b, qs, :].T.astype(BF), wqT], axis=1))
        bigT = np.ascontiguousarray(
            np.concatenate([sK, cK, sQ, cQ], axis=1))
        in_maps.append({
            "bigA": bigA_b[b], "bigE1a": bigE1a, "bigT": bigT,
            "bigE2": wvT, "bigF2": woT,
            "smalls": smalls,
        })

    res = run_bass_kernel_spmd(nc, in_maps, core_ids=list(range(NCORES)))
    kernel._last_result = res

    out = np.empty((B, SQ, D), np.float32)
    for core in range(NCORES):
        b, half = core // 2, core % 2
        out[b, half * SQL:(half + 1) * SQL, :] = \
            res.results[core]["outT"].T.astype(np.float32)
    return out
